# revision 1
# baseline (speedup 1.0000x reference)
"""GCN autoencoder kernel for 8 Trainium2 NeuronCores.

Strategy (self-contained; shapes hardcoded for the graded problem):
  - Nodes row-sharded 1250/core; edge list partitioned by dst and sorted.
  - Per core: Y1 = x_slab @ W1 (PE transposes of x + matmuls), AllGather of the
    row-padded Y1 table, dma_gather of per-edge 256B rows, segment-sum via PE
    matmuls against DVE-built one-hot*weight selection matrices (32-dst column
    strips via tile_position), relu -> H, AllGather, same aggregation again,
    then z^T = W2^T @ zpre^T, AllGather z^T.
  - Decode: out = sigmoid(z_own @ z_all^T) with float32r matmuls (N=512
    chunks), ScalarE sigmoid from PSUM, 5MB/row-tile streaming stores.
"""

from contextlib import ExitStack
from dataclasses import dataclass

import numpy as np

import concourse.bass as bass
import concourse.mybir as mybir
import concourse.tile as tile
from concourse import bacc
from concourse.bass_utils import run_bass_kernel_spmd

dt = mybir.dt


@dataclass
class Cfg:
    n_nodes: int = 10000
    n_feat: int = 512
    hid: int = 32
    code: int = 16
    n_cores: int = 8
    gs: int = 32          # dst nodes per PSUM column strip
    chunk: int = 128      # edges per matmul chunk
    pad: int = 128        # bf16 elements per gather row (256B)
    decode_dt: str = "bfloat16"
    ablate: int = 4       # 1=y1+AG, 2=+layer1, 3=+layer2/zt, 4=full
    n_queues: int = 4     # SWDGE queues for parallel gather desc-gen
    Cg: tuple = ()        # per-group chunk counts (data dependent; from prep)

    @property
    def rows(self):
        return self.n_nodes // self.n_cores

    @property
    def ng(self):  # groups per core
        return -(-self.rows // self.gs)

    @property
    def mt(self):  # 128-row m-tiles per core
        return -(-self.rows // 128)

    @property
    def nch(self):  # chunks per core
        return sum(self.Cg)

    @property
    def kch(self):  # 128-row K chunks of n_feat
        return self.n_feat // 128

    @property
    def chunk_base(self):
        b, acc = [], 0
        for c in self.Cg:
            b.append(acc)
            acc += c
        return b


def prep_edges(cfg: Cfg, src, dst, ew):
    """Sort edges by dst, shard by dst range, group into gs-dst groups each
    padded to C*chunk slots. Returns per-core (gidx int16 [128, nch*chunk/16],
    wt f32 [128, nch], dmb f32 [128, nch]) and the chosen C."""
    src = np.asarray(src).astype(np.int64)
    dst = np.asarray(dst).astype(np.int64)
    ew = np.asarray(ew).astype(np.float32)
    order = np.argsort(dst, kind="stable")
    s_s, d_s, w_s = src[order], dst[order], ew[order]

    per_core = []
    maxcnt = np.zeros(cfg.ng, np.int64)
    for c in range(cfg.n_cores):
        lo = c * cfg.rows
        m = (d_s >= lo) & (d_s < lo + cfg.rows)
        sc, dc, wc = s_s[m], d_s[m] - lo, w_s[m]
        gids = dc // cfg.gs
        counts = np.bincount(gids, minlength=cfg.ng)
        maxcnt = np.maximum(maxcnt, counts)
        per_core.append((sc, dc, wc, counts))
    # per-group chunk count, uniform across cores (program uniformity)
    cfg.Cg = tuple(int(x) for x in np.maximum(1, -(-maxcnt // cfg.chunk)))

    cbase = cfg.chunk_base
    slots = cfg.nch * cfg.chunk
    outs = []
    for sc, dc, wc, counts in per_core:
        srcpad = np.zeros(slots, np.int64)
        wpad = np.zeros(slots, np.float32)
        dmbpad = np.full(slots, -1.0, np.float32)
        pos = 0
        for g in range(cfg.ng):
            cnt = counts[g]
            base = cbase[g] * cfg.chunk
            srcpad[base : base + cnt] = sc[pos : pos + cnt]
            wpad[base : base + cnt] = wc[pos : pos + cnt]
            dmbpad[base : base + cnt] = (dc[pos : pos + cnt] - g * cfg.gs).astype(
                np.float32
            )
            pos += cnt
        gidx16 = srcpad.reshape(-1, 16).T.astype(np.int16)  # [16, slots/16]
        gidx = np.tile(gidx16, (8, 1)).copy()  # [128, slots/16]
        wt = wpad.reshape(cfg.nch, cfg.chunk).T.copy()  # [128, nch]
        dmb = dmbpad.reshape(cfg.nch, cfg.chunk).T.copy()
        outs.append((gidx, wt, dmb))
    return outs


def build_nc(cfg: Cfg):
    nc = bacc.Bacc(
        "TRN2",
        target_bir_lowering=False,
        debug=False,
        enable_asserts=False,
        num_devices=cfg.n_cores,
        num_swdge_queues=cfg.n_queues,
    )
    f32 = dt.float32
    bf16 = dt.bfloat16
    N, R, HID, CODE, PAD = cfg.n_nodes, cfg.rows, cfg.hid, cfg.code, cfg.pad
    GS, CH, NG, MT, KCH = cfg.gs, cfg.chunk, cfg.ng, cfg.mt, cfg.kch
    CG, CBASE = cfg.Cg, cfg.chunk_base
    ddt = getattr(dt, cfg.decode_dt)

    # ---- external I/O ----
    xs = nc.dram_tensor("xs", [R, cfg.n_feat], f32, kind="ExternalInput").ap()
    w1 = nc.dram_tensor("w1", [cfg.n_feat, HID], f32, kind="ExternalInput").ap()
    w2 = nc.dram_tensor("w2", [HID, CODE], f32, kind="ExternalInput").ap()
    ident_d = nc.dram_tensor("ident", [128, 128], f32, kind="ExternalInput").ap()
    iota_d = nc.dram_tensor("iota", [128, GS], f32, kind="ExternalInput").ap()
    gidx_d = nc.dram_tensor(
        "gidx", [128, cfg.nch * CH // 16], dt.int16, kind="ExternalInput"
    ).ap()
    wt_d = nc.dram_tensor("wt", [128, cfg.nch], f32, kind="ExternalInput").ap()
    dmb_d = nc.dram_tensor("dmb", [128, cfg.nch], f32, kind="ExternalInput").ap()
    out_d = nc.dram_tensor("out", [R, N], f32, kind="ExternalOutput").ap()

    # ---- internal DRAM ----
    y1_own = nc.dram_tensor("y1_own", [R, PAD], bf16).ap()
    y1_all = nc.dram_tensor("y1_all", [N, PAD], bf16, addr_space="Shared").ap()
    h_own = nc.dram_tensor("h_own", [R, PAD], bf16).ap()
    h_all = nc.dram_tensor("h_all", [N, PAD], bf16, addr_space="Shared").ap()
    zt_own = nc.dram_tensor("zt_own", [CODE, R], ddt).ap()
    zt_all = nc.dram_tensor(
        "zt_all", [cfg.n_cores, CODE, R], ddt, addr_space="Shared"
    ).ap()

    groups_all = [list(range(cfg.n_cores))]

    def rows_of(m):  # valid rows in m-tile m
        return min(128, R - m * 128)

    def jmax_of(m):  # column strips in m-tile m
        return min(4, NG - 4 * m)

    # decode N-chunking: 512-wide chunks grouped 4 per PSUM tile
    nchunks = []
    n0 = 0
    while n0 < N:
        nn = min(512, N - n0)
        nchunks.append((n0, nn))
        n0 += nn
    bank_groups = [nchunks[i : i + 4] for i in range(0, len(nchunks), 4)]

    # gather call split: whole m-tiles (4 groups) per call
    GPC = 4 if NG % 4 == 0 else NG  # groups per gather call
    NCALL = NG // GPC
    GBW = max(
        CBASE[c * GPC + GPC - 1] + CG[c * GPC + GPC - 1] - CBASE[c * GPC]
        for c in range(NCALL)
    )  # widest call, in chunks

    with tile.TileContext(nc) as tc, ExitStack() as ctx:
        # ---- long-lived pools ----
        cpool = ctx.enter_context(tc.tile_pool(name="consts", bufs=1))
        spool = ctx.enter_context(tc.tile_pool(name="smat", bufs=1))
        zpool = ctx.enter_context(tc.tile_pool(name="zbits", bufs=1))

        # x-path constants first — they gate the Y1 critical path; edge
        # constants (gidx/wt/dmb) aren't needed until the first gather
        ident = cpool.tile([128, 128], f32)
        nc.sync.dma_start(ident[:], ident_d[:, :])
        w1s = cpool.tile([128, KCH, HID], f32)
        for k in range(KCH):
            nc.sync.dma_start(w1s[:, k, :], w1[k * 128 : (k + 1) * 128, :])
        w2s = cpool.tile([HID, CODE], f32)
        nc.sync.dma_start(w2s[:], w2[:, :])
        iota = cpool.tile([128, GS], f32)
        nc.sync.dma_start(iota[:], iota_d[:, :])
        gidx = cpool.tile([128, cfg.nch * CH // 16], dt.int16)
        nc.scalar.dma_start(gidx[:], gidx_d[:, :])
        wts = cpool.tile([128, cfg.nch], f32)
        nc.scalar.dma_start(wts[:], wt_d[:, :])
        dmbs = cpool.tile([128, cfg.nch], f32)
        nc.scalar.dma_start(dmbs[:], dmb_d[:, :])

        smat = spool.tile([128, cfg.nch, GS], bf16)  # selection matrices (reused)
        zts = zpool.tile([CODE, R], ddt)  # own z^T staging
        # decode operands replicated at 4 partition strips (row-grp rotation
        # lets LDWEIGHTS overlap in-flight matmuls)
        zts4 = zpool.tile([128, R], ddt)
        ztall4 = zpool.tile([128, N], ddt)
        zpreT = zpool.tile([HID, MT * 128], f32)

        # ================= phase A/B: x^T and Y1 =================
        with tc.tile_pool(name="xio", bufs=2) as xio, tc.tile_pool(
            name="xt", bufs=1
        ) as xtp, tc.tile_pool(name="pst", bufs=2, space="PSUM") as pst, tc.tile_pool(
            name="psy", bufs=2, space="PSUM"
        ) as psy, tc.tile_pool(name="stage", bufs=2) as stage:
            xT = xtp.tile([128, KCH, MT * 128], f32)
            for m in range(MT):
                rm = rows_of(m)
                xin = xio.tile([128, cfg.n_feat], f32)
                nc.sync.dma_start(xin[:rm, :], xs[m * 128 : m * 128 + rm, :])
                for k in range(KCH):
                    pt = pst.tile([128, 128], f32, space="PSUM")
                    nc.tensor.transpose(
                        pt[:, :rm],
                        xin[:rm, k * 128 : (k + 1) * 128],
                        ident[:rm, :rm],
                    )
                    nc.vector.tensor_copy(
                        xT[:, k, m * 128 : m * 128 + rm], pt[:, :rm]
                    )
            for m in range(MT):
                rm = rows_of(m)
                py = psy.tile([128, HID], f32, space="PSUM")
                for k in range(KCH):
                    nc.tensor.matmul(
                        py[:rm, :],
                        lhsT=xT[:, k, m * 128 : m * 128 + rm],
                        rhs=w1s[:, k, :],
                        start=(k == 0),
                        stop=(k == KCH - 1),
                    )
                st = stage.tile([128, PAD], bf16)
                nc.vector.memset(st[:, HID:PAD], 0.0)
                nc.vector.tensor_copy(st[:rm, 0:HID], py[:rm, :])
                nc.sync.dma_start(y1_own[m * 128 : m * 128 + rm, :], st[:rm, :])

        nc.gpsimd.collective_compute(
            "AllGather",
            mybir.AluOpType.bypass,
            replica_groups=groups_all,
            ins=[y1_own.opt()],
            outs=[y1_all.opt()],
        )

        # ================= SpMM layers =================
        def spmm(src_tab, build_s, emit_group_out, tag):
            with tc.tile_pool(name=f"gbuf_{tag}", bufs=5) as gpool, tc.tile_pool(
                name=f"psg_{tag}", bufs=4, space="PSUM"
            ) as psg:
                for call in range(NCALL):
                    gpc = min(GPC, NG - call * GPC)
                    c0 = CBASE[call * GPC]  # first chunk of this call
                    glast = call * GPC + gpc - 1
                    cpc = CBASE[glast] + CG[glast] - c0  # chunks this call
                    nidx = cpc * CH
                    gb = gpool.tile([128, GBW, PAD], bf16, tag="gb")
                    nc.gpsimd.dma_gather(
                        out_ap=gb[:, :cpc, :],
                        in_ap=src_tab[:, :],
                        idxs_ap=gidx[:, c0 * CH // 16 : (c0 + cpc) * CH // 16],
                        num_idxs=nidx,
                        num_idxs_reg=nidx,
                        elem_size=PAD,
                        single_packet=False,
                        queue_num=call % cfg.n_queues,
                    )
                    for gl in range(gpc):
                        g = call * GPC + gl
                        m, j = divmod(g, 4)
                        if j == 0:
                            pm = psg.tile([128, HID], f32, space="PSUM", tag="pm")
                        for t in range(CG[g]):
                            tg = CBASE[g] + t
                            s_t = smat[:, tg, :]
                            if build_s:
                                nc.vector.tensor_scalar(
                                    s_t,
                                    iota[:, :],
                                    dmbs[:, tg : tg + 1],
                                    wts[:, tg : tg + 1],
                                    op0=mybir.AluOpType.is_equal,
                                    op1=mybir.AluOpType.mult,
                                )
                            nc.tensor.matmul(
                                pm[j * GS : (j + 1) * GS, :],
                                lhsT=s_t,
                                rhs=gb[:, tg - c0, 0:HID],
                                start=(t == 0),
                                stop=(t == CG[g] - 1),
                                tile_position=(0, j * GS),
                            )
                        if j == jmax_of(m) - 1:
                            emit_group_out(m, pm)

        # ---- layer 1: H = relu(A @ Y1), padded + AllGather ----
        if cfg.ablate >= 2:
            with tc.tile_pool(name="hstage", bufs=2) as hstage:

                def l1_out(m, pm):
                    rm = rows_of(m)
                    st = hstage.tile([128, PAD], bf16)
                    nc.vector.memset(st[:, HID:PAD], 0.0)
                    nc.scalar.activation(
                        st[:rm, 0:HID],
                        pm[:rm, :],
                        mybir.ActivationFunctionType.Relu,
                    )
                    nc.sync.dma_start(
                        h_own[m * 128 : m * 128 + rm, :], st[:rm, :]
                    )

                spmm(y1_all, build_s=True, emit_group_out=l1_out, tag="l1")

            nc.gpsimd.collective_compute(
                "AllGather",
                mybir.AluOpType.bypass,
                replica_groups=groups_all,
                ins=[h_own.opt()],
                outs=[h_all.opt()],
            )

        # ---- layer 2: zpre = A @ H, transposed into zpreT ----
        if cfg.ablate >= 3:
            _layer2(tc, nc, cfg, spmm, rows_of, ident, zpreT, w2s, zts,
                    zt_own, zt_all, zts4, ztall4, h_all, groups_all)

        # ================= decode =================
        if cfg.ablate >= 4:
            _decode(tc, nc, cfg, rows_of, bank_groups, zts4, ztall4, out_d)

    nc.compile()
    return nc


def _layer2(tc, nc, cfg, spmm, rows_of, ident, zpreT, w2s, zts, zt_own,
            zt_all, zts4, ztall4, h_all, groups_all):
    f32 = dt.float32
    R, HID, CODE = cfg.rows, cfg.hid, cfg.code
    with tc.tile_pool(name="zstage", bufs=2) as zstage, tc.tile_pool(
        name="pstz", bufs=2, space="PSUM"
    ) as pstz:

        def l2_out(m, pm):
            rm = rows_of(m)
            zp = zstage.tile([128, HID], f32)
            nc.vector.tensor_copy(zp[:rm, :], pm[:rm, :])
            ptz = pstz.tile([HID, 128], f32, space="PSUM")
            nc.tensor.transpose(ptz[:, :rm], zp[:rm, :], ident[:rm, :rm])
            nc.vector.tensor_copy(
                zpreT[:, m * 128 : m * 128 + rm], ptz[:, :rm]
            )

        spmm(h_all, build_s=False, emit_group_out=l2_out, tag="l2")

        # z^T = W2^T @ zpre^T   [CODE, R]
        zn0 = 0
        while zn0 < R:
            zn = min(512, R - zn0)
            pzc = pstz.tile([CODE, 512], f32, space="PSUM", tag="pzc")
            nc.tensor.matmul(
                pzc[:, :zn],
                lhsT=w2s[:, :],
                rhs=zpreT[:, zn0 : zn0 + zn],
                start=True,
                stop=True,
            )
            nc.vector.tensor_copy(zts[:, zn0 : zn0 + zn], pzc[:, :zn])
            zn0 += zn
        nc.sync.dma_start(zt_own[:, :], zts[:, :])

    nc.gpsimd.collective_compute(
        "AllGather",
        mybir.AluOpType.bypass,
        replica_groups=groups_all,
        ins=[zt_own.opt()],
        outs=[zt_all.opt()],
    )
    # load z^T gathered into 4 partition strips: ztall4[32s+p, r*R+j]
    CODE = cfg.code
    for s in range(4):
        nc.sync.dma_start(
            ztall4[32 * s : 32 * s + CODE, :].rearrange(
                "p (r j) -> p r j", r=cfg.n_cores
            ),
            zt_all.rearrange("r p j -> p r j"),
        )
        nc.sync.dma_start(zts4[32 * s : 32 * s + CODE, :], zt_own[:, :])


def _decode(tc, nc, cfg, rows_of, bank_groups, zts4, ztall4, out_d):
    f32 = dt.float32
    N, CODE = cfg.n_nodes, cfg.code
    with tc.tile_pool(name="obuf", bufs=2) as obuf, tc.tile_pool(
        name="psd", bufs=2, space="PSUM"
    ) as psd:
        qq = 0
        for m in range(cfg.mt):
            rm = rows_of(m)
            ob = obuf.tile([128, N], f32)
            for bg in bank_groups:
                # only the last chunk of a group can be short, so the
                # written psum region [0, w) is dense
                w = sum(nn for _, nn in bg)
                pd = psd.tile([128, 2048], f32, space="PSUM")
                for q, (nn0, nn) in enumerate(bg):
                    s = qq % 4  # rotate PE row strips so LDW pipelines
                    qq += 1
                    p0 = 32 * s
                    nc.tensor.matmul(
                        pd[:rm, q * 512 : q * 512 + nn],
                        lhsT=zts4[p0 : p0 + CODE, m * 128 : m * 128 + rm],
                        rhs=ztall4[p0 : p0 + CODE, nn0 : nn0 + nn],
                        start=True,
                        stop=True,
                        tile_position=(p0, 0),
                    )
                b0 = bg[0][0]
                nc.scalar.activation(
                    ob[:rm, b0 : b0 + w],
                    pd[:rm, :w],
                    mybir.ActivationFunctionType.Sigmoid,
                )
            nc.sync.dma_start(out_d[m * 128 : m * 128 + rm, :], ob[:rm, :])


def _host_prep(cfg: Cfg, x, W1, W2, edge_weight, src, dst):
    per_core_edges = prep_edges(cfg, src, dst, edge_weight)
    ident = np.eye(128, dtype=np.float32)
    iota0 = np.tile(np.arange(cfg.gs, dtype=np.float32), (128, 1)).copy()
    in_maps = []
    x = np.ascontiguousarray(np.asarray(x, dtype=np.float32))
    W1 = np.ascontiguousarray(np.asarray(W1, dtype=np.float32))
    W2 = np.ascontiguousarray(np.asarray(W2, dtype=np.float32))
    for c in range(cfg.n_cores):
        gidx, wt, dmb = per_core_edges[c]
        in_maps.append(
            {
                "xs": np.ascontiguousarray(x[c * cfg.rows : (c + 1) * cfg.rows]),
                "w1": W1,
                "w2": W2,
                "ident": ident,
                "iota": iota0,
                "gidx": np.ascontiguousarray(gidx),
                "wt": np.ascontiguousarray(wt),
                "dmb": np.ascontiguousarray(dmb),
            }
        )
    return in_maps


def kernel(x, W1, W2, edge_weight, src, dst, trace=False):
    cfg = Cfg()
    in_maps = _host_prep(cfg, x, W1, W2, edge_weight, src, dst)
    nc = build_nc(cfg)
    res = run_bass_kernel_spmd(
        nc, in_maps, core_ids=list(range(cfg.n_cores)), trace=trace
    )
    out = np.concatenate([r["out"] for r in res.results], axis=0)
    if trace:
        kernel.last_results = res
    return np.ascontiguousarray(out.astype(np.float32))



# revision 7
# speedup vs baseline: 2.1600x; 2.1600x over previous
"""GCN autoencoder kernel for 8 Trainium2 NeuronCores.

Strategy (self-contained; shapes hardcoded for the graded problem):
  - Nodes row-sharded 1250/core. The normalized adjacency slab A^T
    [10112 src, 1250 dst] is host-precomputed in fp8-e4m3 (12.6MB/core),
    DMA'd into SBUF once at startup, and each SpMM layer is a dense PE
    sweep: out^T[feat, dst] = sum_k Y_k^T fp8-stationary @ A^T_k fp8-moving.
  - Per core: Y1 = x_slab @ W1 (PE transposes + matmuls) -> bf16 AllGather,
    cast to fp8 k-tiles; L1 sweep -> relu -> hw2 = h @ W2 -> transpose to
    node-major -> bf16 AllGather -> fp8 k-tiles; L2 sweep -> z^T -> bf16
    AllGather of z^T.
  - Decode: out = sigmoid(z_own @ z_all^T) with bf16 matmuls (N=512 chunks,
    4-strip PE row rotation), ScalarE sigmoid from PSUM, bf16 output rows
    (cast to f32 on host).
"""

from contextlib import ExitStack
from dataclasses import dataclass

import numpy as np
import ml_dtypes

import concourse.bass as bass
import concourse.mybir as mybir
import concourse.tile as tile
from concourse import bacc
from concourse.bass_utils import run_bass_kernel_spmd

dt = mybir.dt


@dataclass
class Cfg:
    n_nodes: int = 10000
    n_feat: int = 512
    hid: int = 32
    code: int = 16
    n_cores: int = 8

    @property
    def rows(self):
        return self.n_nodes // self.n_cores  # 1250

    @property
    def kt(self):  # 128-row k-tiles over the (padded) node dim
        return -(-self.n_nodes // 128)  # 79

    @property
    def npad(self):
        return self.kt * 128  # 10112

    @property
    def mt(self):  # 128-row m-tiles per core
        return -(-self.rows // 128)  # 10

    @property
    def kch(self):  # 128-row K chunks of n_feat
        return self.n_feat // 128  # 4

    @property
    def jchunks(self):  # dst-column chunks of the A^T sweep (psum-bank sized)
        out, j0 = [], 0
        while j0 < self.rows:
            jn = min(512, self.rows - j0)
            out.append((j0, jn))
            j0 += jn
        return out


def build_nc(cfg: Cfg):
    nc = bacc.Bacc(
        "TRN2",
        target_bir_lowering=False,
        debug=False,
        enable_asserts=False,
        num_devices=cfg.n_cores,
    )
    f32 = dt.float32
    bf16 = dt.bfloat16
    fp8 = dt.float8e4
    N, R, HID, CODE = cfg.n_nodes, cfg.rows, cfg.hid, cfg.code
    KT, MT, KCH = cfg.kt, cfg.mt, cfg.kch
    JC = cfg.jchunks

    # ---- external I/O ----
    xs = nc.dram_tensor("xs", [R, cfg.n_feat], f32, kind="ExternalInput").ap()
    w1 = nc.dram_tensor("w1", [cfg.n_feat, HID], f32, kind="ExternalInput").ap()
    w2 = nc.dram_tensor("w2", [HID, CODE], f32, kind="ExternalInput").ap()
    ident_d = nc.dram_tensor("ident", [128, 128], f32, kind="ExternalInput").ap()
    # A^T slab, partition-major: at[p, k*R + j] = A[dst=c*R+j, src=128k+p]
    at_d = nc.dram_tensor("at", [128, KT * R], fp8, kind="ExternalInput").ap()
    out_d = nc.dram_tensor("out", [R, N], bf16, kind="ExternalOutput").ap()

    # ---- internal DRAM ----
    y1_own = nc.dram_tensor("y1_own", [R, HID], bf16).ap()
    y1_all = nc.dram_tensor("y1_all", [N, HID], bf16, addr_space="Shared").ap()
    hw2_own = nc.dram_tensor("hw2_own", [R, CODE], bf16).ap()
    hw2_all = nc.dram_tensor("hw2_all", [N, CODE], bf16, addr_space="Shared").ap()
    zt_own = nc.dram_tensor("zt_own", [CODE, R], bf16).ap()
    zt_all = nc.dram_tensor(
        "zt_all", [cfg.n_cores, CODE, R], bf16, addr_space="Shared"
    ).ap()

    groups_all = [list(range(cfg.n_cores))]

    def rows_of(m):
        return min(128, R - m * 128)

    # decode N-chunking: 512-wide chunks grouped 4 per PSUM tile
    nchunks = []
    n0 = 0
    while n0 < N:
        nn = min(512, N - n0)
        nchunks.append((n0, nn))
        n0 += nn
    bank_groups = [nchunks[i : i + 4] for i in range(0, len(nchunks), 4)]

    with tile.TileContext(nc) as tc, ExitStack() as ctx:
        cpool = ctx.enter_context(tc.tile_pool(name="consts", bufs=1))
        apool = ctx.enter_context(tc.tile_pool(name="amat", bufs=1))
        tabs = ctx.enter_context(tc.tile_pool(name="tabs", bufs=1))
        zpool = ctx.enter_context(tc.tile_pool(name="zbits", bufs=1))

        # A^T resident in SBUF for both layers (98.75KB/partition)
        atile = apool.tile([128, KT, R], fp8)
        nc.sync.dma_start(atile[:].rearrange("p k j -> p (k j)"), at_d[:, :])

        ident = cpool.tile([128, 128], f32)
        nc.sync.dma_start(ident[:], ident_d[:, :])
        w1s = cpool.tile([128, KCH, HID], f32)
        for k in range(KCH):
            nc.scalar.dma_start(w1s[:, k, :], w1[k * 128 : (k + 1) * 128, :])
        w2s = cpool.tile([HID, CODE], f32)
        nc.scalar.dma_start(w2s[:], w2[:, :])

        # fp8 stationary tables (node-major k-tiles) for the two sweeps
        y1k = tabs.tile([128, KT, HID], fp8)
        hk = tabs.tile([128, KT, CODE], fp8)
        # zero the pad rows of the last k-tile once (A^T pad cols are zero
        # too, but keep the stationaries finite)
        nc.vector.memset(y1k[:, KT - 1, :], 0.0)
        nc.vector.memset(hk[:, KT - 1, :], 0.0)

        zts4 = zpool.tile([128, R], bf16)
        ztall4 = zpool.tile([128, N], bf16)

        # ================= phase A: Y1 = x @ W1 =================
        with tc.tile_pool(name="xio", bufs=2) as xio, tc.tile_pool(
            name="xt", bufs=1
        ) as xtp, tc.tile_pool(name="pst", bufs=2, space="PSUM") as pst, tc.tile_pool(
            name="psy", bufs=2, space="PSUM"
        ) as psy, tc.tile_pool(name="stage", bufs=2) as stage:
            xT = xtp.tile([128, KCH, MT * 128], f32)
            for m in range(MT):
                rm = rows_of(m)
                xin = xio.tile([128, cfg.n_feat], f32)
                nc.sync.dma_start(xin[:rm, :], xs[m * 128 : m * 128 + rm, :])
                for k in range(KCH):
                    pt = pst.tile([128, 128], f32, space="PSUM")
                    nc.tensor.transpose(
                        pt[:, :rm],
                        xin[:rm, k * 128 : (k + 1) * 128],
                        ident[:rm, :rm],
                    )
                    nc.vector.tensor_copy(
                        xT[:, k, m * 128 : m * 128 + rm], pt[:, :rm]
                    )
            for m in range(MT):
                rm = rows_of(m)
                py = psy.tile([128, HID], f32, space="PSUM")
                for k in range(KCH):
                    nc.tensor.matmul(
                        py[:rm, :],
                        lhsT=xT[:, k, m * 128 : m * 128 + rm],
                        rhs=w1s[:, k, :],
                        start=(k == 0),
                        stop=(k == KCH - 1),
                    )
                st = stage.tile([128, HID], bf16)
                nc.vector.tensor_copy(st[:rm, :], py[:rm, :])
                nc.sync.dma_start(y1_own[m * 128 : m * 128 + rm, :], st[:rm, :])

        nc.gpsimd.collective_compute(
            "AllGather",
            mybir.AluOpType.bypass,
            replica_groups=groups_all,
            ins=[y1_own.opt()],
            outs=[y1_all.opt()],
        )

        # load gathered table into k-tiles and cast to fp8
        def load_table(dst_fp8, src_dram, width, tag):
            with tc.tile_pool(name=f"tl_{tag}", bufs=1) as tl:
                sb = tl.tile([128, KT, width], bf16)
                nc.vector.memset(sb[:, KT - 1, :], 0.0)
                nc.sync.dma_start(
                    sb[:, 0 : KT - 1, :],
                    src_dram[0 : (KT - 1) * 128, :].rearrange(
                        "(k p) f -> p k f", p=128
                    ),
                )
                nc.sync.dma_start(
                    sb[0 : N - (KT - 1) * 128, KT - 1, :],
                    src_dram[(KT - 1) * 128 : N, :],
                )
                nc.vector.tensor_copy(dst_fp8[:], sb[:])

        load_table(y1k, y1_all, HID, "y1")

        # ================= sweeps =================
        def sweep(stat, width, pse, out_cb):
            """out^T[0:width, j] = sum_k stat[:, k, :].T @ atile[:, k, :]"""
            ps = [
                pse.tile(
                    [width, 512], f32, space="PSUM", name=f"acc{ci}", tag=f"acc{ci}"
                )
                for ci in range(len(JC))
            ]
            for k in range(KT):
                for ci, (j0, jn) in enumerate(JC):
                    nc.tensor.matmul(
                        ps[ci][:, :jn],
                        lhsT=stat[:, k, :],
                        rhs=atile[:, k, j0 : j0 + jn],
                        start=(k == 0),
                        stop=(k == KT - 1),
                    )
            out_cb(ps)

        # ---- layer 1: h^T = relu(A @ Y1)^T, then hw2 = (h @ W2) ----
        with tc.tile_pool(name="hsb", bufs=1) as hsbp, tc.tile_pool(
            name="pse", bufs=1, space="PSUM"
        ) as pse, tc.tile_pool(name="psw", bufs=2, space="PSUM") as psw, tc.tile_pool(
            name="hq", bufs=1
        ) as hqp, tc.tile_pool(name="ptz", bufs=2, space="PSUM") as ptzp:
            hsb = hsbp.tile([HID, R], f32)
            hw2sb = hsbp.tile([CODE, R], f32)
            hw2q = hqp.tile([128, MT, CODE], bf16)

            def l1_out(ps):
                for ci, (j0, jn) in enumerate(JC):
                    nc.scalar.activation(
                        hsb[:, j0 : j0 + jn],
                        ps[ci][:, :jn],
                        mybir.ActivationFunctionType.Relu,
                    )

            sweep(y1k, HID, pse, l1_out)

            # hw2^T = W2^T @ h^T
            for ci, (j0, jn) in enumerate(JC):
                pw = psw.tile([CODE, 512], f32, space="PSUM")
                nc.tensor.matmul(
                    pw[:, :jn],
                    lhsT=w2s[:, :],
                    rhs=hsb[:, j0 : j0 + jn],
                    start=True,
                    stop=True,
                )
                nc.vector.tensor_copy(hw2sb[:, j0 : j0 + jn], pw[:, :jn])

            # transpose to node-major [R, CODE], stage bf16
            for m in range(MT):
                rm = rows_of(m)
                ptz = ptzp.tile([128, CODE], f32, space="PSUM")
                nc.tensor.transpose(
                    ptz[:rm, :],
                    hw2sb[:, m * 128 : m * 128 + rm],
                    ident[:CODE, :CODE],
                )
                nc.vector.tensor_copy(hw2q[:rm, m, :], ptz[:rm, :])
            for m in range(MT):
                rm = rows_of(m)
                nc.sync.dma_start(
                    hw2_own[m * 128 : m * 128 + rm, :], hw2q[:rm, m, :]
                )

        nc.gpsimd.collective_compute(
            "AllGather",
            mybir.AluOpType.bypass,
            replica_groups=groups_all,
            ins=[hw2_own.opt()],
            outs=[hw2_all.opt()],
        )

        load_table(hk, hw2_all, CODE, "hk")

        # ---- layer 2: z^T = (A @ hw2)^T ----
        with tc.tile_pool(name="zsb", bufs=1) as zsbp, tc.tile_pool(
            name="pse2", bufs=1, space="PSUM"
        ) as pse2:
            zts = zsbp.tile([CODE, R], bf16)

            def l2_out(ps):
                for ci, (j0, jn) in enumerate(JC):
                    nc.vector.tensor_copy(zts[:, j0 : j0 + jn], ps[ci][:, :jn])

            sweep(hk, CODE, pse2, l2_out)
            nc.sync.dma_start(zt_own[:, :], zts[:, :])

        nc.gpsimd.collective_compute(
            "AllGather",
            mybir.AluOpType.bypass,
            replica_groups=groups_all,
            ins=[zt_own.opt()],
            outs=[zt_all.opt()],
        )
        # decode operands replicated at 4 partition strips (row-grp rotation
        # lets LDWEIGHTS overlap in-flight matmuls)
        for s in range(4):
            nc.sync.dma_start(
                ztall4[32 * s : 32 * s + CODE, :].rearrange(
                    "p (r j) -> p r j", r=cfg.n_cores
                ),
                zt_all.rearrange("r p j -> p r j"),
            )
            nc.sync.dma_start(zts4[32 * s : 32 * s + CODE, :], zt_own[:, :])

        # ================= decode =================
        with tc.tile_pool(name="obuf", bufs=2) as obuf, tc.tile_pool(
            name="psd", bufs=2, space="PSUM"
        ) as psd:
            qq = 0
            for m in range(MT):
                rm = rows_of(m)
                ob = obuf.tile([128, N], bf16)
                for bg in bank_groups:
                    w = sum(nn for _, nn in bg)
                    pd = psd.tile([128, 2048], f32, space="PSUM")
                    for q, (nn0, nn) in enumerate(bg):
                        s = qq % 4
                        qq += 1
                        p0 = 32 * s
                        nc.tensor.matmul(
                            pd[:rm, q * 512 : q * 512 + nn],
                            lhsT=zts4[p0 : p0 + CODE, m * 128 : m * 128 + rm],
                            rhs=ztall4[p0 : p0 + CODE, nn0 : nn0 + nn],
                            start=True,
                            stop=True,
                            tile_position=(p0, 0),
                        )
                    b0 = bg[0][0]
                    nc.scalar.activation(
                        ob[:rm, b0 : b0 + w],
                        pd[:rm, :w],
                        mybir.ActivationFunctionType.Sigmoid,
                    )
                nc.sync.dma_start(out_d[m * 128 : m * 128 + rm, :], ob[:rm, :])

    nc.compile()
    return nc


def _host_prep(cfg: Cfg, x, W1, W2, edge_weight, src, dst):
    x = np.ascontiguousarray(np.asarray(x, dtype=np.float32))
    W1 = np.ascontiguousarray(np.asarray(W1, dtype=np.float32))
    W2 = np.ascontiguousarray(np.asarray(W2, dtype=np.float32))
    src = np.asarray(src).astype(np.int64)
    dst = np.asarray(dst).astype(np.int64)
    ew = np.asarray(edge_weight).astype(np.float32)
    ident = np.eye(128, dtype=np.float32)

    R, KT = cfg.rows, cfg.kt
    in_maps = []
    for c in range(cfg.n_cores):
        lo = c * R
        m = (dst >= lo) & (dst < lo + R)
        a = np.zeros((cfg.npad, R), np.float32)
        np.add.at(a, (src[m], dst[m] - lo), ew[m])
        # partition-major: at[p, k*R + j] = a[128k + p, j]
        at = (
            a.reshape(KT, 128, R)
            .transpose(1, 0, 2)
            .reshape(128, KT * R)
            .astype(ml_dtypes.float8_e4m3)
        )
        in_maps.append(
            {
                "xs": np.ascontiguousarray(x[lo : lo + R]),
                "w1": W1,
                "w2": W2,
                "ident": ident,
                "at": np.ascontiguousarray(at),
            }
        )
    return in_maps


def kernel(x, W1, W2, edge_weight, src, dst, trace=False):
    cfg = Cfg()
    in_maps = _host_prep(cfg, x, W1, W2, edge_weight, src, dst)
    nc = build_nc(cfg)
    res = run_bass_kernel_spmd(
        nc, in_maps, core_ids=list(range(cfg.n_cores)), trace=trace
    )
    out = np.concatenate([r["out"] for r in res.results], axis=0)
    if trace:
        kernel.last_results = res
    return np.ascontiguousarray(out.astype(np.float32))
